# revision 11
# baseline (speedup 1.0000x reference)
"""Trainium2 Bass kernel for nn_DeChunkLayerReference.

The reference collapses mathematically: with state dim n=1, C==1, B=p and
per-(b,t) scalars shared across all heads, the SSD is a per-channel scalar
EMA along the M=2048 compressed sequence:

    y[b,t,:] = exp(-dt[t]) * y[b,t-1,:] + (p[t]/dt[t]) * hidden[b,t,:]

followed by a gather that duplicates each compressed row to the L=4096
output positions (plug = cumsum(boundary_mask)-1).

Because the state dimension is 1, the carried state IS the output row:
y[T0-1, :].  Chunked (128) computation therefore needs only the DIAGONAL
lower-triangular coefficient block per chunk plus a rank-1 correction:

    y_i = LTdiag_i^T @ x_i  +  v_i (x) y_{i-1}[last row]

with LTdiag[s,t] = exp(cumA[t]-cumA[s]+log w[s]) (s<=t, within chunk) and
v_i[t] = exp(cumA[T0_i + t] - cumA[T0_i - 1]) <= 1.  On device each chunk
is two PE matmuls into one PSUM accumulation (the 128x128 diagonal block,
then a contraction-1 outer product with the previous chunk's last row)
plus one tiny [1,512] row copy on the vector engine to stage the state.
No inter-chunk band matmuls, no data-dependent band count -> one cached
program.

The coefficient blocks depend only on the tiny boundary_prob /
boundary_mask inputs, so they are computed on the host in float64 and
shipped as bf16 (16 diag blocks + 16 v rows ~ 0.52 MiB/core).  hidden is
shipped bf16 in the exact SBUF tile layout (linear DMA), the matmuls run
bf16 (f32 PSUM), and the compressed (M, qw) result is returned bf16; the
host does the rep-2 plug duplication and the f32 upcast.  Per-core HBM
traffic ~4.7 MiB.

Sharding over the 8 cores: (batch b in {0,1}) x (d_model quarter q in
{0..3}); each core processes its full sequence for a 512-wide channel
slice, so there is no cross-core communication at all.
"""

import numpy as np
import ml_dtypes

import concourse.tile as tile
from concourse import bacc, mybir
from concourse.bass_utils import run_bass_kernel_spmd

# Problem shapes (hardcoded per harness contract).
B = 2
M = 2048
D_MODEL = 2048
LFULL = 4096
CHUNK = 128
C = M // CHUNK          # 16 chunks
NCORES = 8
NQ = 4                  # d_model quarters
QW = D_MODEL // NQ      # 512 channels per core
EPS = 1e-4
UFLOW = -103.0          # ln(smallest fp32 denormal) ~ -103.28

GROUP = 4               # chunks per wide x tile
NG = C // GROUP         # 4 groups
PAIR = 2                # chunks per output staging tile / DMA

F32 = mybir.dt.float32
BF16 = mybir.dt.bfloat16
NP_BF16 = ml_dtypes.bfloat16

_prog_cache: dict = {}


def _host_precompute(boundary_mask, boundary_prob):
    """float64 coefficient prep from the small inputs."""
    bm = np.asarray(boundary_mask)
    bp = np.asarray(boundary_prob)
    p = np.clip(bp[..., -1].astype(np.float32), EPS, 1.0 - EPS)
    token_idx = np.arange(bm.shape[1])[None, :] + (~bm).astype(np.int32) * bm.shape[1]
    order = np.argsort(token_idx, axis=1, kind="stable")
    p_sel = np.take_along_axis(p, order[:, :M], axis=1).astype(np.float64)  # (B, M)
    dt = -np.log1p(-p_sel)
    w = p_sel / dt
    logw = np.log(w)
    cumA = np.cumsum(-dt, axis=1)                       # (B, M) inclusive
    plug = np.cumsum(bm.astype(np.int64), axis=1) - 1   # (B, L)
    return logw, cumA, plug


def _build_ltd(cumA, logw):
    """Diagonal LT blocks, bf16, [B, 128, C*128]."""
    lt = np.empty((B, CHUNK, C * CHUNK), NP_BF16)
    smask = np.arange(CHUNK)[:, None] > np.arange(CHUNK)[None, :]  # s > t
    for b in range(B):
        for i in range(C):
            T0 = i * CHUNK
            arg = (cumA[b, T0:T0 + CHUNK][None, :]
                   - cumA[b, T0:T0 + CHUNK][:, None]
                   + logw[b, T0:T0 + CHUNK][:, None])
            blk = np.where(smask, 0.0, np.exp(arg))
            # Rotate time: column p holds time (p-1) mod 128, so the
            # chunk's LAST row lands on PSUM partition 0 (engines cannot
            # base-address partition 127).  Host un-rotates the output.
            lt[b, :, i * CHUNK:(i + 1) * CHUNK] = np.roll(
                blk, 1, axis=1).astype(NP_BF16)
    return lt


def _build_v(cumA):
    """Carry-in decay rows v[i, t] = exp(cumA[T0+t] - cumA[T0-1]); v[0]=0.

    Laid out [1, C*128] so every row sits at partition 0 (matmul lhsT
    base-partition constraint)."""
    v = np.zeros((B, 1, C * CHUNK), NP_BF16)
    for b in range(B):
        for i in range(1, C):
            T0 = i * CHUNK
            v[b, 0, i * CHUNK:(i + 1) * CHUNK] = np.roll(np.exp(
                cumA[b, T0:T0 + CHUNK] - cumA[b, T0 - 1]), 1).astype(NP_BF16)
    return v


def _build_program():
    nc = bacc.Bacc(
        "TRN2", target_bir_lowering=False, debug=False, num_devices=NCORES
    )
    x = nc.dram_tensor("x", [NG * CHUNK, GROUP * QW], BF16, kind="ExternalInput")
    ltd = nc.dram_tensor("lt", [CHUNK, C * CHUNK], BF16, kind="ExternalInput")
    vt = nc.dram_tensor("v", [1, C * CHUNK], BF16, kind="ExternalInput")
    y = nc.dram_tensor("y", [CHUNK, C * QW], BF16, kind="ExternalOutput")

    with tile.TileContext(nc) as tc:
        with tc.tile_pool(name="xp", bufs=1) as xp, \
             tc.tile_pool(name="ltp", bufs=1) as ltp, \
             tc.tile_pool(name="vp", bufs=1) as vp, \
             tc.tile_pool(name="wp", bufs=1) as wp, \
             tc.tile_pool(name="lrp", bufs=1) as lrp, \
             tc.tile_pool(name="yp", bufs=6) as yp, \
             tc.tile_pool(name="wpsp", bufs=1, space="PSUM") as wpsp, \
             tc.tile_pool(name="psp", bufs=3, space="PSUM") as psp:

            # x on the sync HWDGE ring first (the critical-path input),
            # linear in DRAM (host pre-layout); chunk 0 in its own tiny
            # tile for tile-granular readiness.
            xin = x.rearrange("(g p) d -> g p d", p=CHUNK)

            def xslice(c0, c1):
                g, a = divmod(c0, GROUP)
                return xin[g][:, a * QW:(a + c1 - c0) * QW]

            xsegs = [(0, 1), (1, 4), (4, 8), (8, 12), (12, 16)]
            xtile = {}
            for c0, c1 in xsegs:
                t = xp.tile([CHUNK, (c1 - c0) * QW], BF16, tag=f"x{c0}")
                nc.sync.dma_start(out=t[:], in_=xslice(c0, c1))
                for c in range(c0, c1):
                    xtile[c] = (t, c - c0)

            def xview(j):
                t, a = xtile[j]
                return t[:, a * QW:(a + 1) * QW]

            # Coefficients on the scalar ring: diag blocks for chunks 0-3
            # first, the tiny v rows, then the rest.
            lt0 = ltp.tile([CHUNK, 4 * CHUNK], BF16, tag="lt0")
            nc.scalar.dma_start(out=lt0[:], in_=ltd[:, :4 * CHUNK])
            vtile = vp.tile([1, C * CHUNK], BF16, tag="v")
            nc.scalar.dma_start(out=vtile[:], in_=vt[:, :])
            lt1 = ltp.tile([CHUNK, (C - 4) * CHUNK], BF16, tag="lt1")
            nc.scalar.dma_start(out=lt1[:], in_=ltd[:, 4 * CHUNK:])

            def ltview(i):
                if i < 4:
                    return lt0[:, i * CHUNK:(i + 1) * CHUNK]
                return lt1[:, (i - 4) * CHUNK:(i - 3) * CHUNK]

            warm = wp.tile([CHUNK, QW], BF16, tag="warm")
            nc.gpsimd.memset(warm[:], 0.0)

            # PE clock warm-up while the first inputs are in flight.
            wps = wpsp.tile([CHUNK, QW], F32, tag="wps")
            for _ in range(8):
                nc.tensor.matmul(wps[:], lhsT=warm[:, :CHUNK],
                                 rhs=warm[:], start=True, stop=True)

            def bridge(n):
                # Dummy matmuls over input-wait windows so the PE clock
                # never drops out of boost.
                for _ in range(n):
                    nc.tensor.matmul(wps[:], lhsT=warm[:, :CHUNK],
                                     rhs=warm[:], start=True, stop=True)

            # State rows: lr[:, i*QW:(i+1)*QW] = chunk i's DIAG-ONLY last
            # output row (partition 0 in the rotated layout).  The true
            # carry also includes the previous state times the chunk's
            # total decay exp(-sum dt) ~ e^-128, which underflows f32, so
            # the diag-only row IS the carried state -- no serial chain.
            lr = lrp.tile([1, C * QW], BF16, tag="lr")

            # Pipeline per pair h (chunks 2h, 2h+1):
            #   diag x2   PE   psum half = ltdiag^T @ x      [start|stop]
            #   copy      DVE  lr pair  = psum[row 0] (one [1,1024] op)
            #   rank1 x2  PE   psum half += v (x) lr_{i-1}   [stop]
            #   cast      ACT  ypair = bf16(psum)  (one [128,1024] op)
            #   ydma      PL/SYNC alternating rings
            # rank1 runs two pairs behind diag so the DVE copy latency is
            # always covered; element-ops are pair-wide because their cost
            # is free-dim cycles regardless of partition count.
            ps = {}
            yb = {}
            NPAIR = C // PAIR

            def diags(h):
                ps[h] = psp.tile([CHUNK, PAIR * QW], F32, tag="ps",
                                 name=f"ps{h}")
                for ci in range(PAIR):
                    i = h * PAIR + ci
                    nc.tensor.matmul(ps[h][:, ci * QW:(ci + 1) * QW],
                                     lhsT=ltview(i), rhs=xview(i),
                                     start=True, stop=True)
                nc.vector.tensor_copy(
                    lr[:, h * PAIR * QW:(h + 1) * PAIR * QW],
                    ps[h][0:1, :],
                )

            def rank1s(h):
                for ci in range(PAIR):
                    i = h * PAIR + ci
                    if i == 0:
                        continue
                    nc.tensor.matmul(
                        ps[h][:, ci * QW:(ci + 1) * QW],
                        lhsT=vtile[:, i * CHUNK:(i + 1) * CHUNK],
                        rhs=lr[:, (i - 1) * QW:i * QW],
                        start=False, stop=True,
                    )

            def out(h):
                yb[h] = yp.tile([CHUNK, PAIR * QW], BF16, tag="yb",
                                name=f"yb{h}")
                if h == NPAIR - 1:
                    # Final pair: two half casts on both engines in
                    # parallel, two DMAs on two rings -- shortest tail.
                    nc.vector.tensor_copy(yb[h][:, :QW], ps[h][:, :QW])
                    nc.scalar.copy(yb[h][:, QW:], ps[h][:, QW:])
                    nc.gpsimd.dma_start(
                        out=y[:, (C - 2) * QW:(C - 1) * QW],
                        in_=yb[h][:, :QW])
                    nc.sync.dma_start(
                        out=y[:, (C - 1) * QW:C * QW],
                        in_=yb[h][:, QW:])
                else:
                    nc.scalar.copy(yb[h][:], ps[h][:])
                    ring = nc.gpsimd if h % 2 == 0 else nc.sync
                    ring.dma_start(
                        out=y[:, h * PAIR * QW:(h + 1) * PAIR * QW],
                        in_=yb[h][:])

            for h in range(NPAIR):
                diags(h)
                if h == 0:
                    bridge(4)
                elif h == 1:
                    bridge(3)
                elif h == 2:
                    bridge(2)
                if h >= 2:
                    rank1s(h - 2)
                    out(h - 2)
            for h in (NPAIR - 2, NPAIR - 1):
                rank1s(h)
                out(h)
            # Trailing dummies: keep the tensor stream alive past the last
            # real matmul so its end-of-stream drain doesn't delay the
            # completion signals the final casts wait on.
            bridge(4)
    nc.compile()
    return nc


def _run(inputs, trace=False):
    hidden = np.asarray(inputs["hidden_states"], dtype=np.float32)
    logw, cumA, plug = _host_precompute(inputs["boundary_mask"],
                                        inputs["boundary_prob"])

    rep = LFULL // M
    fast = np.array_equal(
        plug, np.tile(np.repeat(np.arange(M), rep)[None, :], (plug.shape[0], 1))
    )
    # Device path drops the chunk-to-chunk state recurrence: the carry
    # into chunk i uses only chunk i-1's local (diag-only) last row,
    # valid because each chunk's total decay exp(-sum dt) underflows f32.
    # Guard that in f64 and fall back if the data ever violates it.
    last = cumA[:, CHUNK - 1::CHUNK]                    # (B, C) chunk-end cumA
    chunk_decay = np.exp(np.diff(last, axis=1)).max() if C > 1 else 0.0
    if not fast or chunk_decay > 1e-25:
        return _numpy_fallback(hidden, logw, cumA, plug), None

    if "prog" not in _prog_cache:
        _prog_cache["prog"] = _build_program()
    nc = _prog_cache["prog"]

    lt_np = _build_ltd(cumA, logw)
    v_np = _build_v(cumA)

    in_maps = []
    for c in range(NCORES):
        b, q = divmod(c, NQ)
        xq = hidden[b, :, q * QW:(q + 1) * QW]
        xq = (xq.reshape(NG, GROUP, CHUNK, QW)
                .transpose(0, 2, 1, 3)
                .reshape(NG * CHUNK, GROUP * QW))
        in_maps.append({
            "x": np.ascontiguousarray(xq.astype(NP_BF16)),
            "lt": lt_np[b],
            "v": v_np[b],
        })

    res = run_bass_kernel_spmd(nc, in_maps, list(range(NCORES)), trace=trace)
    out = np.empty((B, LFULL, D_MODEL), np.float32)
    out4 = out.reshape(B, M, rep, D_MODEL)
    for c in range(NCORES):
        b, q = divmod(c, NQ)
        yc = np.asarray(res.results[c]["y"])          # (128, C*QW) bf16
        t = (np.roll(yc, -1, axis=0)                  # un-rotate time
               .reshape(CHUNK, C, QW)
               .transpose(1, 0, 2)
               .reshape(M, QW)
               .astype(np.float32))
        out4[b, :, :, q * QW:(q + 1) * QW] = t[:, None, :]
    return out, res


def _numpy_fallback(hidden, logw, cumA, plug):
    """Exact CPU path for plug patterns the device program doesn't cover."""
    y = np.zeros((B, M, D_MODEL), np.float32)
    for b in range(B):
        for i in range(C):
            T0 = i * CHUNK
            acc = np.zeros((CHUNK, D_MODEL), np.float64)
            for j in range(i + 1):
                S0 = j * CHUNK
                arg = (cumA[b, T0:T0 + CHUNK][None, :]
                       - cumA[b, S0:S0 + CHUNK][:, None]
                       + logw[b, S0:S0 + CHUNK][:, None])
                if j == i:
                    s_idx = np.arange(CHUNK)
                    arg = np.where(s_idx[:, None] > s_idx[None, :], -np.inf, arg)
                if arg.max() < UFLOW:
                    continue
                LT = np.exp(arg)
                acc += LT.T @ hidden[b, S0:S0 + CHUNK].astype(np.float64)
            y[b, T0:T0 + CHUNK] = acc.astype(np.float32)
    return np.take_along_axis(y, plug[:, :, None].astype(np.int64), axis=1)


def kernel(**inputs) -> np.ndarray:
    out, _ = _run(inputs, trace=False)
    return out


# revision 12
# speedup vs baseline: 1.1792x; 1.1792x over previous
"""Trainium2 Bass kernel for nn_DeChunkLayerReference.

The reference collapses mathematically: with state dim n=1, C==1, B=p and
per-(b,t) scalars shared across all heads, the SSD is a per-channel scalar
EMA along the M=2048 compressed sequence:

    y[b,t,:] = exp(-dt[t]) * y[b,t-1,:] + (p[t]/dt[t]) * hidden[b,t,:]

followed by a gather that duplicates each compressed row to the L=4096
output positions (plug = cumsum(boundary_mask)-1).

Because the state dimension is 1, the carried state IS the output row:
y[T0-1, :].  Chunked (128) computation therefore needs only the DIAGONAL
lower-triangular coefficient block per chunk plus a rank-1 correction:

    y_i = LTdiag_i^T @ x_i  +  v_i (x) y_{i-1}[last row]

with LTdiag[s,t] = exp(cumA[t]-cumA[s]+log w[s]) (s<=t, within chunk) and
v_i[t] = exp(cumA[T0_i + t] - cumA[T0_i - 1]) <= 1.  On device each chunk
is two PE matmuls into one PSUM accumulation (the 128x128 diagonal block,
then a contraction-1 outer product with the previous chunk's last row)
plus one tiny [1,512] row copy on the vector engine to stage the state.
No inter-chunk band matmuls, no data-dependent band count -> one cached
program.

The coefficient blocks depend only on the tiny boundary_prob /
boundary_mask inputs, so they are computed on the host in float64 and
shipped as bf16 (16 diag blocks + 16 v rows ~ 0.52 MiB/core).  hidden is
shipped bf16 in the exact SBUF tile layout (linear DMA), the matmuls run
bf16 (f32 PSUM), and the compressed (M, qw) result is returned bf16; the
host does the rep-2 plug duplication and the f32 upcast.  Per-core HBM
traffic ~4.7 MiB.

Sharding over the 8 cores: (batch b in {0,1}) x (d_model quarter q in
{0..3}); each core processes its full sequence for a 512-wide channel
slice, so there is no cross-core communication at all.
"""

import numpy as np
import ml_dtypes

import concourse.tile as tile
from concourse import bacc, mybir
from concourse.bass_utils import run_bass_kernel_spmd

# Problem shapes (hardcoded per harness contract).
B = 2
M = 2048
D_MODEL = 2048
LFULL = 4096
CHUNK = 128
C = M // CHUNK          # 16 chunks
NCORES = 8
NQ = 4                  # d_model quarters
QW = D_MODEL // NQ      # 512 channels per core
EPS = 1e-4
UFLOW = -103.0          # ln(smallest fp32 denormal) ~ -103.28

GROUP = 4               # chunks per wide x tile
NG = C // GROUP         # 4 groups
PAIR = 2                # chunks per output staging tile / DMA

F32 = mybir.dt.float32
BF16 = mybir.dt.bfloat16
NP_BF16 = ml_dtypes.bfloat16

_prog_cache: dict = {}


def _host_precompute(boundary_mask, boundary_prob):
    """float64 coefficient prep from the small inputs."""
    bm = np.asarray(boundary_mask)
    bp = np.asarray(boundary_prob)
    p = np.clip(bp[..., -1].astype(np.float32), EPS, 1.0 - EPS)
    token_idx = np.arange(bm.shape[1])[None, :] + (~bm).astype(np.int32) * bm.shape[1]
    order = np.argsort(token_idx, axis=1, kind="stable")
    p_sel = np.take_along_axis(p, order[:, :M], axis=1).astype(np.float64)  # (B, M)
    dt = -np.log1p(-p_sel)
    w = p_sel / dt
    logw = np.log(w)
    cumA = np.cumsum(-dt, axis=1)                       # (B, M) inclusive
    plug = np.cumsum(bm.astype(np.int64), axis=1) - 1   # (B, L)
    return logw, cumA, plug


def _build_ltd(cumA, logw):
    """Diagonal LT blocks, bf16, [B, 128, C*128]."""
    lt = np.empty((B, CHUNK, C * CHUNK), NP_BF16)
    smask = np.arange(CHUNK)[:, None] > np.arange(CHUNK)[None, :]  # s > t
    for b in range(B):
        for i in range(C):
            T0 = i * CHUNK
            arg = (cumA[b, T0:T0 + CHUNK][None, :]
                   - cumA[b, T0:T0 + CHUNK][:, None]
                   + logw[b, T0:T0 + CHUNK][:, None])
            blk = np.where(smask, 0.0, np.exp(arg))
            # Rotate time: column p holds time (p-1) mod 128, so the
            # chunk's LAST row lands on PSUM partition 0 (engines cannot
            # base-address partition 127).  Host un-rotates the output.
            lt[b, :, i * CHUNK:(i + 1) * CHUNK] = np.roll(
                blk, 1, axis=1).astype(NP_BF16)
    return lt


def _build_v(cumA):
    """Carry-in decay rows v[i, t] = exp(cumA[T0+t] - cumA[T0-1]); v[0]=0.

    Laid out [1, C*128] so every row sits at partition 0 (matmul lhsT
    base-partition constraint)."""
    v = np.zeros((B, 1, C * CHUNK), NP_BF16)
    for b in range(B):
        for i in range(1, C):
            T0 = i * CHUNK
            v[b, 0, i * CHUNK:(i + 1) * CHUNK] = np.roll(np.exp(
                cumA[b, T0:T0 + CHUNK] - cumA[b, T0 - 1]), 1).astype(NP_BF16)
    return v


def _build_program():
    nc = bacc.Bacc(
        "TRN2", target_bir_lowering=False, debug=False, num_devices=NCORES
    )
    x = nc.dram_tensor("x", [NG * CHUNK, GROUP * QW], BF16, kind="ExternalInput")
    ltd = nc.dram_tensor("lt", [CHUNK, C * CHUNK], BF16, kind="ExternalInput")
    vt = nc.dram_tensor("v", [1, C * CHUNK], BF16, kind="ExternalInput")
    y = nc.dram_tensor("y", [CHUNK, C * QW], BF16, kind="ExternalOutput")

    with tile.TileContext(nc) as tc:
        with tc.tile_pool(name="xp", bufs=1) as xp, \
             tc.tile_pool(name="ltp", bufs=1) as ltp, \
             tc.tile_pool(name="vp", bufs=1) as vp, \
             tc.tile_pool(name="wp", bufs=1) as wp, \
             tc.tile_pool(name="lrp", bufs=1) as lrp, \
             tc.tile_pool(name="yp", bufs=8) as yp, \
             tc.tile_pool(name="wpsp", bufs=1, space="PSUM") as wpsp, \
             tc.tile_pool(name="psp", bufs=6, space="PSUM") as psp:

            # x on the sync HWDGE ring first (the critical-path input),
            # linear in DRAM (host pre-layout); chunk 0 in its own tiny
            # tile for tile-granular readiness.
            xin = x.rearrange("(g p) d -> g p d", p=CHUNK)

            def xslice(c0, c1):
                g, a = divmod(c0, GROUP)
                return xin[g][:, a * QW:(a + c1 - c0) * QW]

            xsegs = [(0, 1), (1, 4), (4, 8), (8, 12), (12, 16)]
            xtile = {}
            for c0, c1 in xsegs:
                t = xp.tile([CHUNK, (c1 - c0) * QW], BF16, tag=f"x{c0}")
                nc.sync.dma_start(out=t[:], in_=xslice(c0, c1))
                for c in range(c0, c1):
                    xtile[c] = (t, c - c0)

            def xview(j):
                t, a = xtile[j]
                return t[:, a * QW:(a + 1) * QW]

            # Coefficients on the scalar ring, early so their descriptors
            # win engine arbitration before the bulk of x.
            lt0 = ltp.tile([CHUNK, 4 * CHUNK], BF16, tag="lt0")
            nc.scalar.dma_start(out=lt0[:], in_=ltd[:, :4 * CHUNK])
            lt1 = ltp.tile([CHUNK, (C - 4) * CHUNK], BF16, tag="lt1")
            nc.scalar.dma_start(out=lt1[:], in_=ltd[:, 4 * CHUNK:])
            vtile = vp.tile([1, C * CHUNK], BF16, tag="v")
            nc.scalar.dma_start(out=vtile[:], in_=vt[:, :])

            def ltview(i):
                if i < 4:
                    return lt0[:, i * CHUNK:(i + 1) * CHUNK]
                return lt1[:, (i - 4) * CHUNK:(i - 3) * CHUNK]

            warm = wp.tile([CHUNK, QW], BF16, tag="warm")
            nc.gpsimd.memset(warm[:], 0.0)

            # PE clock warm-up while the first inputs are in flight.
            wps = wpsp.tile([CHUNK, QW], F32, tag="wps")
            for _ in range(8):
                nc.tensor.matmul(wps[:], lhsT=warm[:, :CHUNK],
                                 rhs=warm[:], start=True, stop=True)

            def bridge(n):
                # Dummy matmuls over input-wait windows so the PE clock
                # never drops out of boost.
                for _ in range(n):
                    nc.tensor.matmul(wps[:], lhsT=warm[:, :CHUNK],
                                     rhs=warm[:], start=True, stop=True)

            # State rows: lr[:, i*QW:(i+1)*QW] = chunk i's DIAG-ONLY last
            # output row (partition 0 in the rotated layout).  The true
            # carry also includes the previous state times the chunk's
            # total decay exp(-sum dt) ~ e^-128, which underflows f32, so
            # the diag-only row IS the carried state -- no serial chain.
            lr = lrp.tile([1, C * QW], BF16, tag="lr")

            # Per-chunk pipeline, per-chunk PSUM tiles (one bank, six in
            # flight) so no stage backpressures another:
            #   diag(i)  PE   psum_i = ltdiag_i^T @ x_i       [start|stop]
            #   copy(i)  DVE/ACT  lr_i = psum_i[row 0] (bf16)
            #   rank1(i) PE   psum_i += v_i (x) lr_{i-1}      [stop]
            #   cast(i)  ACT/DVE  ychunk = bf16(psum_i)
            #   ydma(i)  PL/SYNC alternating rings
            # rank1 runs two chunks behind diag so the copy latency is
            # always covered.
            ps = {}
            yb = {}

            def diag(i):
                ps[i] = psp.tile([CHUNK, QW], F32, tag="pc", name=f"pc{i}")
                nc.tensor.matmul(ps[i][:], lhsT=ltview(i), rhs=xview(i),
                                 start=True, stop=True)

            def copy(i):
                dst = lr[:, i * QW:(i + 1) * QW]
                if i % 2 == 1:
                    nc.vector.tensor_copy(dst, ps[i][0:1, :])
                else:
                    nc.scalar.copy(dst, ps[i][0:1, :])

            def rank1(i):
                nc.tensor.matmul(ps[i][:],
                                 lhsT=vtile[:, i * CHUNK:(i + 1) * CHUNK],
                                 rhs=lr[:, (i - 1) * QW:i * QW],
                                 start=False, stop=True)

            def cast_out(i):
                yb[i] = yp.tile([CHUNK, QW], BF16, tag="yc", name=f"yc{i}")
                if i % 2 == 0:
                    nc.vector.tensor_copy(yb[i][:], ps[i][:])
                else:
                    nc.scalar.copy(yb[i][:], ps[i][:])
                ring = nc.gpsimd if i % 2 == 0 else nc.sync
                ring.dma_start(out=y[:, i * QW:(i + 1) * QW], in_=yb[i][:])

            for i in range(C):
                diag(i)
                if i < C - 1:
                    copy(i)
                if i == 1:
                    bridge(4)
                elif i == 4:
                    bridge(3)
                elif i == 7:
                    bridge(2)
                j = i - 2
                if j >= 0:
                    if j >= 1:
                        rank1(j)
                    cast_out(j)
            for j in (C - 2, C - 1):
                rank1(j)
                cast_out(j)
            # Trailing dummies: keep the tensor stream alive through the
            # end-of-kernel barrier (which waits on the last output DMA
            # completions) so the PE clock is still boosted when the
            # framework postamble's semaphore-reset cascade runs on the
            # tensor queue -- at idle clock those resets are ~2x slower.
            bridge(14)
    nc.compile()
    return nc


def _run(inputs, trace=False):
    hidden = np.asarray(inputs["hidden_states"], dtype=np.float32)
    logw, cumA, plug = _host_precompute(inputs["boundary_mask"],
                                        inputs["boundary_prob"])

    rep = LFULL // M
    fast = np.array_equal(
        plug, np.tile(np.repeat(np.arange(M), rep)[None, :], (plug.shape[0], 1))
    )
    # Device path drops the chunk-to-chunk state recurrence: the carry
    # into chunk i uses only chunk i-1's local (diag-only) last row,
    # valid because each chunk's total decay exp(-sum dt) underflows f32.
    # Guard that in f64 and fall back if the data ever violates it.
    last = cumA[:, CHUNK - 1::CHUNK]                    # (B, C) chunk-end cumA
    chunk_decay = np.exp(np.diff(last, axis=1)).max() if C > 1 else 0.0
    if not fast or chunk_decay > 1e-25:
        return _numpy_fallback(hidden, logw, cumA, plug), None

    if "prog" not in _prog_cache:
        _prog_cache["prog"] = _build_program()
    nc = _prog_cache["prog"]

    lt_np = _build_ltd(cumA, logw)
    v_np = _build_v(cumA)

    in_maps = []
    for c in range(NCORES):
        b, q = divmod(c, NQ)
        xq = hidden[b, :, q * QW:(q + 1) * QW]
        xq = (xq.reshape(NG, GROUP, CHUNK, QW)
                .transpose(0, 2, 1, 3)
                .reshape(NG * CHUNK, GROUP * QW))
        in_maps.append({
            "x": np.ascontiguousarray(xq.astype(NP_BF16)),
            "lt": lt_np[b],
            "v": v_np[b],
        })

    res = run_bass_kernel_spmd(nc, in_maps, list(range(NCORES)), trace=trace)
    out = np.empty((B, LFULL, D_MODEL), np.float32)
    out4 = out.reshape(B, M, rep, D_MODEL)
    for c in range(NCORES):
        b, q = divmod(c, NQ)
        yc = np.asarray(res.results[c]["y"])          # (128, C*QW) bf16
        t = (np.roll(yc, -1, axis=0)                  # un-rotate time
               .reshape(CHUNK, C, QW)
               .transpose(1, 0, 2)
               .reshape(M, QW)
               .astype(np.float32))
        out4[b, :, :, q * QW:(q + 1) * QW] = t[:, None, :]
    return out, res


def _numpy_fallback(hidden, logw, cumA, plug):
    """Exact CPU path for plug patterns the device program doesn't cover."""
    y = np.zeros((B, M, D_MODEL), np.float32)
    for b in range(B):
        for i in range(C):
            T0 = i * CHUNK
            acc = np.zeros((CHUNK, D_MODEL), np.float64)
            for j in range(i + 1):
                S0 = j * CHUNK
                arg = (cumA[b, T0:T0 + CHUNK][None, :]
                       - cumA[b, S0:S0 + CHUNK][:, None]
                       + logw[b, S0:S0 + CHUNK][:, None])
                if j == i:
                    s_idx = np.arange(CHUNK)
                    arg = np.where(s_idx[:, None] > s_idx[None, :], -np.inf, arg)
                if arg.max() < UFLOW:
                    continue
                LT = np.exp(arg)
                acc += LT.T @ hidden[b, S0:S0 + CHUNK].astype(np.float64)
            y[b, T0:T0 + CHUNK] = acc.astype(np.float32)
    return np.take_along_axis(y, plug[:, :, None].astype(np.int64), axis=1)


def kernel(**inputs) -> np.ndarray:
    out, _ = _run(inputs, trace=False)
    return out


# revision 13
# speedup vs baseline: 1.1889x; 1.0082x over previous
"""Trainium2 Bass kernel for nn_DeChunkLayerReference.

The reference collapses mathematically: with state dim n=1, C==1, B=p and
per-(b,t) scalars shared across all heads, the SSD is a per-channel scalar
EMA along the M=2048 compressed sequence:

    y[b,t,:] = exp(-dt[t]) * y[b,t-1,:] + (p[t]/dt[t]) * hidden[b,t,:]

followed by a gather that duplicates each compressed row to the L=4096
output positions (plug = cumsum(boundary_mask)-1).

Because the state dimension is 1, the carried state IS the output row:
y[T0-1, :].  Chunked (128) computation therefore needs only the DIAGONAL
lower-triangular coefficient block per chunk plus a rank-1 correction:

    y_i = LTdiag_i^T @ x_i  +  v_i (x) y_{i-1}[last row]

with LTdiag[s,t] = exp(cumA[t]-cumA[s]+log w[s]) (s<=t, within chunk) and
v_i[t] = exp(cumA[T0_i + t] - cumA[T0_i - 1]) <= 1.  On device each chunk
is two PE matmuls into one PSUM accumulation (the 128x128 diagonal block,
then a contraction-1 outer product with the previous chunk's last row)
plus one tiny [1,512] row copy on the vector engine to stage the state.
No inter-chunk band matmuls, no data-dependent band count -> one cached
program.

The coefficient blocks depend only on the tiny boundary_prob /
boundary_mask inputs, so they are computed on the host in float64 and
shipped as bf16 (16 diag blocks + 16 v rows ~ 0.52 MiB/core).  hidden is
shipped bf16 in the exact SBUF tile layout (linear DMA), the matmuls run
bf16 (f32 PSUM), and the compressed (M, qw) result is returned bf16; the
host does the rep-2 plug duplication and the f32 upcast.  Per-core HBM
traffic ~4.7 MiB.

Sharding over the 8 cores: (batch b in {0,1}) x (d_model quarter q in
{0..3}); each core processes its full sequence for a 512-wide channel
slice, so there is no cross-core communication at all.
"""

import numpy as np
import ml_dtypes

import concourse.tile as tile
from concourse import bacc, mybir
from concourse.bass_utils import run_bass_kernel_spmd

# Problem shapes (hardcoded per harness contract).
B = 2
M = 2048
D_MODEL = 2048
LFULL = 4096
CHUNK = 128
C = M // CHUNK          # 16 chunks
NCORES = 8
NQ = 4                  # d_model quarters
QW = D_MODEL // NQ      # 512 channels per core
EPS = 1e-4
UFLOW = -103.0          # ln(smallest fp32 denormal) ~ -103.28

GROUP = 4               # chunks per wide x tile
NG = C // GROUP         # 4 groups
PAIR = 2                # chunks per output staging tile / DMA

F32 = mybir.dt.float32
BF16 = mybir.dt.bfloat16
NP_BF16 = ml_dtypes.bfloat16

_prog_cache: dict = {}


def _host_precompute(boundary_mask, boundary_prob):
    """float64 coefficient prep from the small inputs."""
    bm = np.asarray(boundary_mask)
    bp = np.asarray(boundary_prob)
    p = np.clip(bp[..., -1].astype(np.float32), EPS, 1.0 - EPS)
    token_idx = np.arange(bm.shape[1])[None, :] + (~bm).astype(np.int32) * bm.shape[1]
    order = np.argsort(token_idx, axis=1, kind="stable")
    p_sel = np.take_along_axis(p, order[:, :M], axis=1).astype(np.float64)  # (B, M)
    dt = -np.log1p(-p_sel)
    w = p_sel / dt
    logw = np.log(w)
    cumA = np.cumsum(-dt, axis=1)                       # (B, M) inclusive
    plug = np.cumsum(bm.astype(np.int64), axis=1) - 1   # (B, L)
    return logw, cumA, plug


def _build_ltd(cumA, logw):
    """Diagonal LT blocks, bf16, [B, 128, C*128]."""
    lt = np.empty((B, CHUNK, C * CHUNK), NP_BF16)
    smask = np.arange(CHUNK)[:, None] > np.arange(CHUNK)[None, :]  # s > t
    for b in range(B):
        for i in range(C):
            T0 = i * CHUNK
            arg = (cumA[b, T0:T0 + CHUNK][None, :]
                   - cumA[b, T0:T0 + CHUNK][:, None]
                   + logw[b, T0:T0 + CHUNK][:, None])
            blk = np.where(smask, 0.0, np.exp(arg))
            # Rotate time: column p holds time (p-1) mod 128, so the
            # chunk's LAST row lands on PSUM partition 0 (engines cannot
            # base-address partition 127).  Host un-rotates the output.
            lt[b, :, i * CHUNK:(i + 1) * CHUNK] = np.roll(
                blk, 1, axis=1).astype(NP_BF16)
    return lt


def _build_v(cumA):
    """Carry-in decay rows v[i, t] = exp(cumA[T0+t] - cumA[T0-1]); v[0]=0.

    Laid out [1, C*128] so every row sits at partition 0 (matmul lhsT
    base-partition constraint)."""
    v = np.zeros((B, 1, C * CHUNK), NP_BF16)
    for b in range(B):
        for i in range(1, C):
            T0 = i * CHUNK
            v[b, 0, i * CHUNK:(i + 1) * CHUNK] = np.roll(np.exp(
                cumA[b, T0:T0 + CHUNK] - cumA[b, T0 - 1]), 1).astype(NP_BF16)
    return v


def _build_program():
    nc = bacc.Bacc(
        "TRN2", target_bir_lowering=False, debug=False, num_devices=NCORES
    )
    x = nc.dram_tensor("x", [NG * CHUNK, GROUP * QW], BF16, kind="ExternalInput")
    ltd = nc.dram_tensor("lt", [CHUNK, C * CHUNK], BF16, kind="ExternalInput")
    vt = nc.dram_tensor("v", [1, C * CHUNK], BF16, kind="ExternalInput")
    y = nc.dram_tensor("y", [CHUNK, C * QW], BF16, kind="ExternalOutput")

    with tile.TileContext(nc) as tc:
        with tc.tile_pool(name="xp", bufs=1) as xp, \
             tc.tile_pool(name="ltp", bufs=1) as ltp, \
             tc.tile_pool(name="vp", bufs=1) as vp, \
             tc.tile_pool(name="wp", bufs=1) as wp, \
             tc.tile_pool(name="lrp", bufs=1) as lrp, \
             tc.tile_pool(name="yp", bufs=8) as yp, \
             tc.tile_pool(name="wpsp", bufs=1, space="PSUM") as wpsp, \
             tc.tile_pool(name="psp", bufs=6, space="PSUM") as psp:

            # x on the sync HWDGE ring first (the critical-path input),
            # linear in DRAM (host pre-layout); chunk 0 in its own tiny
            # tile for tile-granular readiness.
            xin = x.rearrange("(g p) d -> g p d", p=CHUNK)

            def xslice(c0, c1):
                g, a = divmod(c0, GROUP)
                return xin[g][:, a * QW:(a + c1 - c0) * QW]

            xsegs = [(0, 1), (1, 4), (4, 8), (8, 12), (12, 16)]
            xtile = {}
            for c0, c1 in xsegs:
                t = xp.tile([CHUNK, (c1 - c0) * QW], BF16, tag=f"x{c0}")
                nc.sync.dma_start(out=t[:], in_=xslice(c0, c1))
                for c in range(c0, c1):
                    xtile[c] = (t, c - c0)

            def xview(j):
                t, a = xtile[j]
                return t[:, a * QW:(a + 1) * QW]

            # Coefficients on the scalar ring, early so their descriptors
            # win engine arbitration before the bulk of x.
            lt0 = ltp.tile([CHUNK, 4 * CHUNK], BF16, tag="lt0")
            nc.scalar.dma_start(out=lt0[:], in_=ltd[:, :4 * CHUNK])
            lt1 = ltp.tile([CHUNK, (C - 4) * CHUNK], BF16, tag="lt1")
            nc.scalar.dma_start(out=lt1[:], in_=ltd[:, 4 * CHUNK:])
            vtile = vp.tile([1, C * CHUNK], BF16, tag="v")
            nc.scalar.dma_start(out=vtile[:], in_=vt[:, :])

            def ltview(i):
                if i < 4:
                    return lt0[:, i * CHUNK:(i + 1) * CHUNK]
                return lt1[:, (i - 4) * CHUNK:(i - 3) * CHUNK]

            warm = wp.tile([CHUNK, QW], BF16, tag="warm")
            nc.gpsimd.memset(warm[:], 0.0)

            # PE clock warm-up while the first inputs are in flight.
            wps = wpsp.tile([CHUNK, QW], F32, tag="wps")
            for _ in range(8):
                nc.tensor.matmul(wps[:], lhsT=warm[:, :CHUNK],
                                 rhs=warm[:], start=True, stop=True)

            def bridge(n):
                # Dummy matmuls over input-wait windows so the PE clock
                # never drops out of boost.
                for _ in range(n):
                    nc.tensor.matmul(wps[:], lhsT=warm[:, :CHUNK],
                                     rhs=warm[:], start=True, stop=True)

            # State rows: lr[:, i*QW:(i+1)*QW] = chunk i's DIAG-ONLY last
            # output row (partition 0 in the rotated layout).  The true
            # carry also includes the previous state times the chunk's
            # total decay exp(-sum dt) ~ e^-128, which underflows f32, so
            # the diag-only row IS the carried state -- no serial chain.
            lr = lrp.tile([1, C * QW], BF16, tag="lr")

            # Per-chunk pipeline, per-chunk PSUM tiles (one bank, six in
            # flight) so no stage backpressures another:
            #   diag(i)  PE   psum_i = ltdiag_i^T @ x_i       [start|stop]
            #   copy(i)  DVE/ACT  lr_i = psum_i[row 0] (bf16)
            #   rank1(i) PE   psum_i += v_i (x) lr_{i-1}      [stop]
            #   cast(i)  ACT/DVE  ychunk = bf16(psum_i)
            #   ydma(i)  PL/SYNC alternating rings
            # rank1 runs two chunks behind diag so the copy latency is
            # always covered.
            ps = {}
            yb = {}

            def diag(i):
                ps[i] = psp.tile([CHUNK, QW], F32, tag="pc", name=f"pc{i}")
                nc.tensor.matmul(ps[i][:], lhsT=ltview(i), rhs=xview(i),
                                 start=True, stop=True)

            def copy(i):
                # All carry copies on the scalar engine, all output casts
                # on the vector engine: homogeneous per-engine streams
                # pipeline without head-of-line blocking.
                nc.scalar.copy(lr[:, i * QW:(i + 1) * QW], ps[i][0:1, :])

            def rank1(i):
                nc.tensor.matmul(ps[i][:],
                                 lhsT=vtile[:, i * CHUNK:(i + 1) * CHUNK],
                                 rhs=lr[:, (i - 1) * QW:i * QW],
                                 start=False, stop=True)

            def cast_out(i):
                yb[i] = yp.tile([CHUNK, QW], BF16, tag="yc", name=f"yc{i}")
                nc.vector.tensor_copy(yb[i][:], ps[i][:])
                ring = nc.gpsimd if i % 2 == 0 else nc.sync
                ring.dma_start(out=y[:, i * QW:(i + 1) * QW], in_=yb[i][:])

            for i in range(C):
                diag(i)
                if i < C - 1:
                    copy(i)
                if i == 1:
                    bridge(4)
                j = i - 2
                if j >= 0:
                    if j >= 1:
                        rank1(j)
                    cast_out(j)
            for j in (C - 2, C - 1):
                rank1(j)
                cast_out(j)
            # Trailing dummies: keep the tensor stream alive through the
            # end-of-kernel barrier (which waits on the last output DMA
            # completions) so the PE clock is still boosted when the
            # framework postamble's semaphore-reset cascade runs on the
            # tensor queue -- at idle clock those resets are ~2x slower.
            bridge(4)
    nc.compile()
    return nc


def _run(inputs, trace=False):
    hidden = np.asarray(inputs["hidden_states"], dtype=np.float32)
    logw, cumA, plug = _host_precompute(inputs["boundary_mask"],
                                        inputs["boundary_prob"])

    rep = LFULL // M
    fast = np.array_equal(
        plug, np.tile(np.repeat(np.arange(M), rep)[None, :], (plug.shape[0], 1))
    )
    # Device path drops the chunk-to-chunk state recurrence: the carry
    # into chunk i uses only chunk i-1's local (diag-only) last row,
    # valid because each chunk's total decay exp(-sum dt) underflows f32.
    # Guard that in f64 and fall back if the data ever violates it.
    last = cumA[:, CHUNK - 1::CHUNK]                    # (B, C) chunk-end cumA
    chunk_decay = np.exp(np.diff(last, axis=1)).max() if C > 1 else 0.0
    if not fast or chunk_decay > 1e-25:
        return _numpy_fallback(hidden, logw, cumA, plug), None

    if "prog" not in _prog_cache:
        _prog_cache["prog"] = _build_program()
    nc = _prog_cache["prog"]

    lt_np = _build_ltd(cumA, logw)
    v_np = _build_v(cumA)

    in_maps = []
    for c in range(NCORES):
        b, q = divmod(c, NQ)
        xq = hidden[b, :, q * QW:(q + 1) * QW]
        xq = (xq.reshape(NG, GROUP, CHUNK, QW)
                .transpose(0, 2, 1, 3)
                .reshape(NG * CHUNK, GROUP * QW))
        in_maps.append({
            "x": np.ascontiguousarray(xq.astype(NP_BF16)),
            "lt": lt_np[b],
            "v": v_np[b],
        })

    res = run_bass_kernel_spmd(nc, in_maps, list(range(NCORES)), trace=trace)
    out = np.empty((B, LFULL, D_MODEL), np.float32)
    out4 = out.reshape(B, M, rep, D_MODEL)
    for c in range(NCORES):
        b, q = divmod(c, NQ)
        yc = np.asarray(res.results[c]["y"])          # (128, C*QW) bf16
        t = (np.roll(yc, -1, axis=0)                  # un-rotate time
               .reshape(CHUNK, C, QW)
               .transpose(1, 0, 2)
               .reshape(M, QW)
               .astype(np.float32))
        out4[b, :, :, q * QW:(q + 1) * QW] = t[:, None, :]
    return out, res


def _numpy_fallback(hidden, logw, cumA, plug):
    """Exact CPU path for plug patterns the device program doesn't cover."""
    y = np.zeros((B, M, D_MODEL), np.float32)
    for b in range(B):
        for i in range(C):
            T0 = i * CHUNK
            acc = np.zeros((CHUNK, D_MODEL), np.float64)
            for j in range(i + 1):
                S0 = j * CHUNK
                arg = (cumA[b, T0:T0 + CHUNK][None, :]
                       - cumA[b, S0:S0 + CHUNK][:, None]
                       + logw[b, S0:S0 + CHUNK][:, None])
                if j == i:
                    s_idx = np.arange(CHUNK)
                    arg = np.where(s_idx[:, None] > s_idx[None, :], -np.inf, arg)
                if arg.max() < UFLOW:
                    continue
                LT = np.exp(arg)
                acc += LT.T @ hidden[b, S0:S0 + CHUNK].astype(np.float64)
            y[b, T0:T0 + CHUNK] = acc.astype(np.float32)
    return np.take_along_axis(y, plug[:, :, None].astype(np.int64), axis=1)


def kernel(**inputs) -> np.ndarray:
    out, _ = _run(inputs, trace=False)
    return out
